# revision 3
# baseline (speedup 1.0000x reference)
"""Trainium2 Bass kernel for CubeFaceNN.

Computes, for x of shape [8, 1, 128, 128, 128] (f32):
    out[b, i, p] = relu(x[b, 0, p] - x[b, 0, p + OFF[i]])   (zero padded)
with OFF = [(0,-1,-1), (-1,0,-1), (1,-1,-1), (-1,1,-1), (-1,-1,0), (-1,-1,1)]
(derived from the reference's adj % 3 - 1 indexing).

Sharding: pure data parallel — batch b -> NeuronCore b (8 cores).

Per-core layout: depth d on the 128 SBUF partitions, (h, w) in the free
dims. x is resident in SBUF; the partition-shifted copy xp[d] = x[d+1]
(needed by the five od != 0 channels, via the substituted frame
out[i, d'+1] = relu(xp[d'] - x[d', h+oh, w+ow])) is built ON-CHIP by the
idle PE array with a one-subdiagonal shift matrix (exact: one-hot rows),
instead of re-reading 8 MiB from HBM.

Output is computed in f32 (exact subtract) and rounded once to fp16, which
halves store traffic (48 -> 24 MiB/core); a single fp16 rounding keeps
per-element relative error <= 2^-11, far inside the 2e-2 gate.

Engine budget per core (measured rates on this silicon):
  - SWDGE DMA: 16 engines; stores ~19.6 GB/s/engine at 8-32 KiB
    descriptors, loads ~14.6; total ~33.6 MB moved -> ~110 us floor.
  - DVE: subs as FLAT contiguous APs (a 127-wide row AP halves DVE rate);
    boundary columns/rows are patched afterwards from strip views.
  - Work split so neither DVE nor ACT exceeds ~65 us: DVE = 6 subs +
    relu+strips for 3 channels; ACT = 32 PSUM->SBUF xp copies + relu+strips
    for the other 3 + the d-boundary planes.
  - The channel loop runs in 12 h-half units (4 och buffers) so sub ->
    relu -> store pipelines 4 deep instead of stalling 3-stage x 2-buf.
"""

import numpy as np

import concourse.bacc as bacc
import concourse.mybir as mybir
import concourse.tile as tile
from concourse.bass_utils import run_bass_kernel_spmd

D = H = W = 128
HW = H * W
HALF = 64
UH = 64  # unit = h-half
UF = UH * W
N_CORES = 8
LC = 16  # load chunk rows (8 KiB descriptors)
MMF = 512  # matmul moving free size (one PSUM bank of f32)
F32 = mybir.dt.float32
F16 = mybir.dt.float16

# (od, oh, ow) per output channel
OFFSETS = [(0, -1, -1), (-1, 0, -1), (1, -1, -1), (-1, 1, -1), (-1, -1, 0), (-1, -1, 1)]
DVE_RELU_CHANNELS = (0, 1, 2)  # relu+strips on DVE; rest on ACT

_NC_CACHE = {}


def build_nc(debug=False):
    nc = bacc.Bacc("TRN2", target_bir_lowering=False, debug=debug)
    x = nc.dram_tensor("x", [D, H, W], F32, kind="ExternalInput")
    out = nc.dram_tensor("out", [6, D, H, W], F16, kind="ExternalOutput")
    # shift matrix: sh[k, m] = 1 iff k == m+1, so (sh.T @ x)[m] = x[m+1]
    sh_dram = nc.inline_tensor(np.eye(D, k=-1, dtype=np.float32), name="shift")

    sub = mybir.AluOpType.subtract
    relu = mybir.ActivationFunctionType.Relu

    with tile.TileContext(nc) as tc:
        with (
            tc.tile_pool(name="xt", bufs=1) as xt_pool,
            tc.tile_pool(name="xp", bufs=1) as xp_pool,
            tc.tile_pool(name="sh", bufs=1) as sh_pool,
            tc.tile_pool(name="och", bufs=4) as och_pool,
            tc.tile_pool(name="pf32", bufs=2) as pf32_pool,
            tc.tile_pool(name="pf16", bufs=2) as pf16_pool,
            tc.tile_pool(name="ps", bufs=4, space="PSUM") as ps_pool,
        ):
            sht = sh_pool.tile([D, D], F32)
            nc.sync.dma_start(out=sht[:], in_=sh_dram[:])

            # x resident; (partition-half x 16-row chunk) -> 8 KiB descriptors
            xt = xt_pool.tile([D, H, W], F32)
            for c in range(H // LC):
                hsl = slice(c * LC, (c + 1) * LC)
                nc.gpsimd.dma_start(out=xt[0:HALF, hsl], in_=x[0:HALF, hsl])
                nc.gpsimd.dma_start(out=xt[HALF:D, hsl], in_=x[HALF:D, hsl])
            xt2 = xt.rearrange("d h w -> d (h w)")

            # d-boundary planes (HWDGE ring + ACT, off the 16 SWDGE engines):
            # out[i, 0] = relu(x[0]) for od=-1 channels, out[2, 127] = relu(x[127])
            p0s = pf32_pool.tile([H, W], F32)
            p0 = pf16_pool.tile([H, W], F16)
            nc.sync.dma_start(out=p0s[:], in_=x[0])
            nc.scalar.activation(p0[:], p0s[:], relu)
            p1s = pf32_pool.tile([H, W], F32)
            p1 = pf16_pool.tile([H, W], F16)
            nc.sync.dma_start(out=p1s[:], in_=x[D - 1])
            nc.scalar.activation(p1[:], p1s[:], relu)
            for i, (od, _, _) in enumerate(OFFSETS):
                if od == -1:
                    nc.sync.dma_start(out=out[i, 0], in_=p0[:])
            nc.sync.dma_start(out=out[2, D - 1], in_=p1[:])

            # xp[d] = x[d+1] via PE shift matmul (f32, exact), PSUM -> SBUF on ACT
            xp = xp_pool.tile([D, H, W], F32)
            xp2 = xp.rearrange("d h w -> d (h w)")
            for c in range(HW // MMF):
                fsl = slice(c * MMF, (c + 1) * MMF)
                ps = ps_pool.tile([D, MMF], F32)
                nc.tensor.matmul(
                    out=ps[:], lhsT=sht[:], rhs=xt2[:, fsl], start=True, stop=True
                )
                nc.scalar.copy(out=xp2[:, fsl], in_=ps[:])

            def emit_unit(i, u):
                od, oh, ow = OFFSETS[i]
                dc = D if od == 0 else D - 1
                delta = oh * W + ow
                A3 = xp if od == -1 else xt  # aligned with the output frame
                S2 = xp2 if od == 1 else xt2  # d-shifted operand
                A2 = A3.rearrange("d h w -> d (h w)")

                hs, he = max(0, -oh), H - max(0, oh)
                f0, f1 = u * UF, (u + 1) * UF
                lo = max(f0, -delta)
                hi = min(f1, HW - delta)

                och = och_pool.tile([D, UH, W], F16)
                och2 = och.rearrange("d h w -> d (h w)")
                nc.vector.tensor_tensor(
                    out=och2[0:dc, lo - f0 : hi - f0],
                    in0=A2[0:dc, lo:hi],
                    in1=S2[0:dc, lo + delta : hi + delta],
                    op=sub,
                )
                # strips: shifted source is zero-padding there -> relu(A)
                eng = nc.vector if i in DVE_RELU_CHANNELS else None

                def strip(osel, asel):
                    if eng is not None:
                        eng.tensor_scalar_max(och[osel], A3[asel], 0.0)
                    else:
                        nc.scalar.activation(och[osel], A3[asel], relu)

                r0 = u * UH
                if oh == -1 and u == 0:
                    strip((slice(0, dc), slice(0, 1)), (slice(0, dc), slice(0, 1)))
                if oh == 1 and u == 1:
                    strip(
                        (slice(0, dc), slice(UH - 1, UH)),
                        (slice(0, dc), slice(H - 1, H)),
                    )
                if ow != 0:
                    wb = 0 if ow == -1 else W - 1
                    rs, re = max(hs, r0), min(he, r0 + UH)
                    strip(
                        (slice(0, dc), slice(rs - r0, re - r0), slice(wb, wb + 1)),
                        (slice(0, dc), slice(rs, re), slice(wb, wb + 1)),
                    )
                # interior relu (in place, fp16)
                if eng is not None:
                    eng.tensor_scalar_max(
                        och2[0:dc, lo - f0 : hi - f0], och2[0:dc, lo - f0 : hi - f0], 0.0
                    )
                else:
                    nc.scalar.activation(
                        och2[0:dc, lo - f0 : hi - f0], och2[0:dc, lo - f0 : hi - f0], relu
                    )
                # store: 16 KiB per-partition descriptors, even/odd halves
                d0 = 1 if od == -1 else 0
                rsl = slice(r0, r0 + UH)
                nc.gpsimd.dma_start(out=out[i, d0 : d0 + HALF, rsl], in_=och[0:HALF])
                nc.gpsimd.dma_start(
                    out=out[i, d0 + HALF : d0 + dc, rsl], in_=och[HALF:dc]
                )

            # xpA-independent / early-xp units first, xpB-dependent units last
            for i in range(6):
                emit_unit(i, 0)
            for i in range(6):
                emit_unit(i, 1)

    nc.compile()
    return nc


def _get_nc():
    if "nc" not in _NC_CACHE:
        _NC_CACHE["nc"] = build_nc()
    return _NC_CACHE["nc"]


def kernel(x: np.ndarray) -> np.ndarray:
    assert x.shape == (N_CORES, 1, D, H, W), x.shape
    nc = _get_nc()
    in_maps = [{"x": np.ascontiguousarray(x[b, 0], dtype=np.float32)} for b in range(N_CORES)]
    res = run_bass_kernel_spmd(nc, in_maps, core_ids=list(range(N_CORES)))
    return np.stack(
        [np.asarray(r["out"], dtype=np.float32) for r in res.results], axis=0
    )
